# revision 6
# baseline (speedup 1.0000x reference)
"""Cross-attention kernel for 8 Trainium2 NeuronCores (Bass/Tile). v2

Sharding: data-parallel over (L, query-half). Core c handles batch l = c//2
and queries [(c%2)*1024, (c%2+1)*1024) of that batch. K/V for the full 2048
keys of batch l are computed on both cores of the pair (duplicated work, no
cross-core communication). Each core's x arrives with its own query half
permuted to keys 0..1023 (softmax/AV are permutation-invariant over keys;
the mask bias is permuted to match), so q-proj reads the first NQ columns.

Per-core dataflow (matmuls in bf16, f32 PSUM accumulation):
  kT[e, nk], qT[e, nq]    projections, feature dim on partitions
  v[nk, h, 65]            v projection + ones column per head (softmax denom)
  sT[nk, 1024]            scores for a (head-pair, query-512-half): the two
                          heads' K=64 matmuls run concurrently as PE row
                          tiles (base partitions 0/64) into col halves
  at = exp(SCALE*sT + maskbias[nk])   one ACT op [128,1024] per key chunk
  AV via attn-as-weights: out[128q, 65] += at_chunk[:, tslice].T @ v[c,h,:]
      (K=128 keys, M=128 queries -> FWL weight loads keep full MAC rate;
       col 64 accumulates the softmax denominator)
  normalize: [128,4] reciprocal + per-partition tensor_scalar multiplies
  O^T assembled by PE transposes; out = O^T.T @ WoT + bo per 128-query tile
"""

import numpy as np
import ml_dtypes
from contextlib import ExitStack

import concourse.bass as bass
import concourse.tile as tile
from concourse import bacc, mybir
from concourse.bass_utils import run_bass_kernel_spmd

L, N, D_IN = 4, 2048, 1024
H, DH = 8, 64
INNER = H * DH          # 512
D_OUT = D_IN
SCALE = DH ** -0.5      # 0.125
NQ = N // 2             # 1024 queries per core
NCORES = 8
DC = D_IN // 128        # 8 contraction chunks for the projections
NP = H // 2             # 4 head pairs
KC = N // 128           # 16 key chunks
LAG = 6                 # AV c-groups trail their scores by LAG key chunks
MASK_NEG = -50.0

BF = mybir.dt.bfloat16
F32 = mybir.dt.float32
EXP = mybir.ActivationFunctionType.Exp

# phase order: qh-major so the qh=0 output projections overlap qh=1 phases
PHASES = [(p, qh) for qh in (0, 1) for p in range(NP)]


def _emit(ctx, tc, xT, wqT, wkT, wvT, woT, bo, maskb, ident, out):
    nc = tc.nc

    const = ctx.enter_context(tc.tile_pool(name="const", bufs=1))
    big = ctx.enter_context(tc.tile_pool(name="big", bufs=1))
    atp = ctx.enter_context(tc.tile_pool(name="atp", bufs=12))
    nrm = ctx.enter_context(tc.tile_pool(name="nrm", bufs=3))
    outp = ctx.enter_context(tc.tile_pool(name="outp", bufs=2))
    ps_st = ctx.enter_context(tc.tile_pool(name="ps_st", bufs=2, space="PSUM"))
    ps_ch = ctx.enter_context(tc.tile_pool(name="ps_ch", bufs=2, space="PSUM"))
    ps_av = ctx.enter_context(tc.tile_pool(name="ps_av", bufs=2, space="PSUM"))

    # ---- inputs -> SBUF. Weight layouts are partition-major; xT loads per
    # (d-chunk, key-half) with the own-query half first so projections can
    # start before the full input lands.
    wk_s = const.tile([128, DC, INNER], BF)
    wq_s = const.tile([128, DC, INNER], BF)
    wv_s = const.tile([128, DC, INNER], BF)
    wo_s = const.tile([128, NP, D_OUT], BF)
    bo_s = const.tile([1, D_OUT], F32)
    maskb_s = const.tile([128, KC], F32)
    id_s = const.tile([128, 128], BF)
    xT_s = big.tile([128, DC, N], BF)
    nc.sync.dma_start(wk_s, wkT)
    for d in range(DC):
        nc.sync.dma_start(xT_s[:, d, 0:NQ], xT[d][:, 0:NQ])
    nc.sync.dma_start(wq_s, wqT)
    nc.sync.dma_start(maskb_s, maskb)
    for d in range(DC):
        nc.sync.dma_start(xT_s[:, d, NQ:N], xT[d][:, NQ:N])
    nc.sync.dma_start(wv_s, wvT)
    nc.sync.dma_start(wo_s, woT)
    nc.sync.dma_start(bo_s, bo)
    nc.sync.dma_start(id_s, ident)

    ones_row = const.tile([1, 128], BF)
    nc.vector.memset(ones_row, 1.0)
    bo_bf = const.tile([1, D_OUT], BF)
    nc.vector.tensor_copy(bo_bf, bo_s)

    kT_s = big.tile([128, NP, N], BF)
    qT_s = big.tile([128, NP, NQ], BF)
    v_sb = big.tile([128, KC, H, DH + 1], BF)
    nc.vector.memset(v_sb[:, :, :, DH], 1.0)
    o_sb = big.tile([128, 8, H, DH], BF)      # [q, t_global, h, dh]
    oT_sb = big.tile([128, NP, NQ], BF)       # [inner-of-pair, p, q]

    # ---- warmup: junk matmuls lift the PE HAM clock gate and a junk exp
    # pulls the ACT table load off the critical path, all during DMA.
    warm = const.tile([128, 512], BF)
    nc.vector.memset(warm, 1.0)
    wps = ps_av.tile([128, 512], F32, tag="av", name="wps")
    for i in range(18):
        nc.tensor.matmul(wps, warm[:, 0:128], warm, start=(i == 0),
                         stop=(i == 17))
    warm_out = const.tile([1, 32], BF)
    nc.scalar.activation(warm_out, wps[0:1, 0:32], EXP, bias=0.0, scale=0.0)

    # ---- projection / outproj groups (PE fill work between attention ops)
    def proj_kT(p, kh):
        ps = ps_st.tile([128, 1024], F32, tag="st", name=f"ps_k{p}{kh}")
        for d in range(DC):
            for half in range(2):
                nc.tensor.matmul(
                    ps[:, half * 512:(half + 1) * 512],
                    wk_s[:, d, p * 128:(p + 1) * 128],
                    xT_s[:, d, kh * 1024 + half * 512:kh * 1024 + (half + 1) * 512],
                    start=(d == 0), stop=(d == DC - 1))
        nc.vector.tensor_copy(kT_s[:, p, kh * 1024:(kh + 1) * 1024], ps)

    def proj_qT(p):
        ps = ps_st.tile([128, 1024], F32, tag="st", name=f"ps_q{p}")
        for d in range(DC):
            for half in range(2):
                nc.tensor.matmul(
                    ps[:, half * 512:(half + 1) * 512],
                    wq_s[:, d, p * 128:(p + 1) * 128],
                    xT_s[:, d, half * 512:(half + 1) * 512],
                    start=(d == 0), stop=(d == DC - 1))
        nc.vector.tensor_copy(qT_s[:, p, :], ps)

    def proj_v(c):
        ps = ps_av.tile([128, 512], F32, tag="av", name=f"ps_v{c}")
        for d in range(DC):
            nc.tensor.matmul(
                ps, xT_s[:, d, c * 128:(c + 1) * 128], wv_s[:, d, :],
                start=(d == 0), stop=(d == DC - 1))
        nc.vector.tensor_copy(
            v_sb[:, c, :, 0:DH],
            ps.rearrange("p (h e) -> p h e", h=H))

    def outproj(t):
        ps = ps_st.tile([128, 1024], F32, tag="st", name=f"ps_o{t}")
        for half in range(2):
            hs = slice(half * 512, (half + 1) * 512)
            nc.tensor.matmul(ps[:, hs], ones_row, bo_bf[:, hs],
                             start=True, stop=False)
            for p in range(NP):
                nc.tensor.matmul(ps[:, hs],
                                 oT_sb[:, p, t * 128:(t + 1) * 128],
                                 wo_s[:, p, hs], start=False,
                                 stop=(p == NP - 1))
        of = outp.tile([128, D_OUT], F32, tag="of", name="of")
        nc.vector.tensor_copy(of, ps)
        nc.sync.dma_start(out[t * 128:(t + 1) * 128, :], of)

    # ---- attention machinery
    at_tiles = [[None] * KC for _ in range(8)]
    av_chains = [None] * 8

    def scores(k, c):
        p, qh = PHASES[k]
        sT = ps_st.tile([128, 1024], F32, tag="st", name="sT")
        nc.tensor.matmul(
            sT[:, 0:512], kT_s[0:64, p, c * 128:(c + 1) * 128],
            qT_s[0:64, p, qh * 512:(qh + 1) * 512], start=True, stop=True)
        nc.tensor.matmul(
            sT[:, 512:1024], kT_s[64:128, p, c * 128:(c + 1) * 128],
            qT_s[64:128, p, qh * 512:(qh + 1) * 512], start=True, stop=True)
        att = atp.tile([128, 1024], BF, tag="at", name="at")
        nc.scalar.activation(att, sT, EXP, bias=maskb_s[:, c:c + 1],
                             scale=SCALE)
        at_tiles[k][c] = att

    def av_group(k, j):
        # chain (hl, tl) accumulates attn^T.T @ [v|1] for 128 queries
        p, qh = PHASES[k]
        avA, avB = av_chains[k]
        att = at_tiles[k][j]
        for hl in range(2):
            ps = avA if hl == 0 else avB
            for tl in range(4):
                # start zeroes the whole 2KB bank (all 4 chains); per-element
                # first-touch overwrites handle the other chains' first writes
                nc.tensor.matmul(
                    ps[:, tl * 65:tl * 65 + 65],
                    att[:, hl * 512 + tl * 128:hl * 512 + (tl + 1) * 128],
                    v_sb[:, j, 2 * p + hl, :],
                    start=(j == 0 and tl == 0),
                    stop=(j == KC - 1 and tl == 3))
        at_tiles[k][j] = None

    def norm_tr(k):
        # normalize by the accumulated denominators (col 64 of each chain)
        # and transpose O blocks into oT for the output projection
        p, qh = PHASES[k]
        avA, avB = av_chains[k]
        rA = nrm.tile([128, 4], F32, tag="rA", name="rA")
        rB = nrm.tile([128, 4], F32, tag="rB", name="rB")
        nc.vector.reciprocal(rA, avA.rearrange("q (t x) -> q t x", x=65)[:, :, 64])
        nc.vector.reciprocal(rB, avB.rearrange("q (t x) -> q t x", x=65)[:, :, 64])
        for tl in range(4):
            tg = qh * 4 + tl
            nc.vector.tensor_scalar_mul(
                o_sb[:, tg, 2 * p, :], avA[:, tl * 65:tl * 65 + 64],
                rA[:, tl:tl + 1])
            nc.vector.tensor_scalar_mul(
                o_sb[:, tg, 2 * p + 1, :], avB[:, tl * 65:tl * 65 + 64],
                rB[:, tl:tl + 1])
        trc = ps_av.tile([128, 512], BF, tag="av", name="trc")
        for tl in range(4):
            tg = qh * 4 + tl
            nc.tensor.transpose(trc[:, tl * 128:(tl + 1) * 128],
                                o_sb[:, tg, 2 * p:2 * p + 2, :], id_s)
        nc.vector.tensor_copy(oT_sb[:, p, qh * 512:(qh + 1) * 512], trc)

    # ---- fill queue drained inside the attention loop
    fill_q = [("k", 0, 1), ("v", 0, 0), ("v", 1, 0), ("v", 2, 0),
              ("v", 3, 0), ("v", 4, 0), ("v", 5, 0), ("v", 6, 0),
              ("k", 1, 0), ("v", 7, 0), ("v", 8, 0), ("v", 9, 0),
              ("k", 1, 1), ("v", 10, 0), ("v", 11, 0), ("q", 1, 0),
              ("v", 12, 0), ("v", 13, 0), ("v", 14, 0), ("v", 15, 0),
              ("k", 2, 0), ("k", 2, 1), ("q", 2, 0),
              ("k", 3, 0), ("k", 3, 1), ("q", 3, 0)]

    def pop_fill():
        kind, a, b = fill_q.pop(0)
        if kind == "k":
            proj_kT(a, b)
        elif kind == "q":
            proj_qT(a)
        elif kind == "v":
            proj_v(a)
        else:
            outproj(a)

    proj_kT(0, 0)
    proj_qT(0)

    for k in range(8):
        avA = ps_ch.tile([128, 260], F32, tag="ch", name=f"avA{k}")
        avB = ps_ch.tile([128, 260], F32, tag="ch", name=f"avB{k}")
        av_chains[k] = (avA, avB)
        for c in range(KC):
            scores(k, c)
            if c < LAG and k >= 1:
                av_group(k - 1, KC - LAG + c)
                if c == LAG - 1:
                    norm_tr(k - 1)
            if c - LAG >= 0:
                av_group(k, c - LAG)
            if k == 4 and c == LAG + 1:
                fill_q.extend([("o", t, 0) for t in range(4)])
            if fill_q:
                # drain roughly evenly; front-load during the first phases
                budget = 2 if k == 0 else 1
                for _ in range(budget):
                    if fill_q:
                        pop_fill()

    # ---- tail: last phase's trailing AV groups, normalize, outproj qh=1
    for j in range(KC - LAG, KC):
        av_group(7, j)
    norm_tr(7)
    while fill_q:
        pop_fill()
    for t in range(4, 8):
        outproj(t)


def _build():
    nc = bacc.Bacc("TRN2", target_bir_lowering=False, debug=False,
                   num_devices=NCORES)
    aps = dict(
        xT=nc.dram_tensor("xT", [DC, 128, N], BF, kind="ExternalInput").ap(),
        wqT=nc.dram_tensor("wqT", [128, DC, INNER], BF, kind="ExternalInput").ap(),
        wkT=nc.dram_tensor("wkT", [128, DC, INNER], BF, kind="ExternalInput").ap(),
        wvT=nc.dram_tensor("wvT", [128, DC, INNER], BF, kind="ExternalInput").ap(),
        woT=nc.dram_tensor("woT", [128, NP, D_OUT], BF, kind="ExternalInput").ap(),
        bo=nc.dram_tensor("bo", [1, D_OUT], F32, kind="ExternalInput").ap(),
        maskb=nc.dram_tensor("maskb", [128, KC], F32, kind="ExternalInput").ap(),
        ident=nc.dram_tensor("ident", [128, 128], BF, kind="ExternalInput").ap(),
        out=nc.dram_tensor("out", [NQ, D_OUT], F32, kind="ExternalOutput").ap(),
    )
    with tile.TileContext(nc) as tc:
        with ExitStack() as ctx:
            _emit(ctx, tc, **aps)
    nc.compile()
    return nc


_prog = None


def _get_prog():
    global _prog
    if _prog is None:
        _prog = _build()
    return _prog


def _make_in_maps(x, Wq, Wk, Wv, Wo, bo, mask):
    bf = ml_dtypes.bfloat16
    f32 = np.float32

    def wlayout(w, chunks):
        # [out, in] -> partition-major [128, chunks, out]
        t = np.asarray(w).T.astype(bf).reshape(chunks, 128, w.shape[0])
        return np.ascontiguousarray(t.transpose(1, 0, 2))

    wqT = wlayout(Wq, DC)
    wkT = wlayout(Wk, DC)
    wvT = wlayout(Wv, DC)
    woT = wlayout(Wo, NP)
    bo2 = np.ascontiguousarray(bo).astype(f32).reshape(1, D_OUT)
    ident = np.eye(128, dtype=bf)
    in_maps = []
    for c in range(NCORES):
        l, qh = c // 2, c % 2
        # key order per core: own query half first (q proj reads cols 0..NQ)
        perm = np.r_[qh * NQ:(qh + 1) * NQ, (1 - qh) * NQ:(2 - qh) * NQ]
        xTl = np.ascontiguousarray(x[l][perm].T.astype(bf).reshape(DC, 128, N))
        mb = np.where(mask[l][perm], 0.0, MASK_NEG).astype(f32)
        mb = np.ascontiguousarray(mb.reshape(KC, 128).T)
        in_maps.append(dict(xT=xTl, wqT=wqT, wkT=wkT, wvT=wvT,
                            woT=woT, bo=bo2, maskb=mb, ident=ident))
    return in_maps


def run(x, Wq, Wk, Wv, Wo, bo, mask, trace=False, tmpdir=None):
    nc = _get_prog()
    in_maps = _make_in_maps(x, Wq, Wk, Wv, Wo, bo, mask)
    res = run_bass_kernel_spmd(nc, in_maps, core_ids=list(range(NCORES)),
                               trace=trace, tmpdir=tmpdir)
    out = np.empty((L, N, D_OUT), np.float32)
    for c in range(NCORES):
        l, qh = c // 2, c % 2
        out[l, qh * NQ:(qh + 1) * NQ, :] = res.results[c]["out"]
    return out, res


def kernel(x, Wq, Wk, Wv, Wo, bo, mask):
    out, _ = run(np.asarray(x, np.float32), np.asarray(Wq, np.float32),
                 np.asarray(Wk, np.float32), np.asarray(Wv, np.float32),
                 np.asarray(Wo, np.float32), np.asarray(bo, np.float32),
                 np.asarray(mask))
    return out


# revision 10
# speedup vs baseline: 1.2412x; 1.2412x over previous
"""Cross-attention kernel for 8 Trainium2 NeuronCores (Bass/Tile). v3

Sharding: data-parallel over (L, query-half). Core c handles batch l = c//2
and queries [(c%2)*1024, (c%2+1)*1024) of that batch. K/V for the full 2048
keys of batch l are computed on both cores of the pair (duplicated work, no
cross-core communication). Each core's x arrives with its own query half
permuted to keys 0..1023, so q-proj reads the first NQ columns.

Engine plan: the kernel is jointly PE/ACT-limited (~155us of work each), so
the emission interleaves everything at fine grain:
  - scores sT[keys128, 1024] per (pair, qh, chunk): two K=64 matmuls run
    concurrently as PE row tiles; one [128,1024] exp on ACT (bias=mask).
  - AV attn-as-weights: out[128q, 65] += at[:, tslice].T @ [v|1]; K=M=128
    keeps FWL weight loads pipelined (~37ns/matmul measured); col 64
    accumulates the softmax denominator. Groups trail scores by LAG chunks
    and spill into the next phase.
  - normalization: [128,4] reciprocals + per-partition tensor_scalar mults.
  - O^T via PE transposes; output projection per 128-query tile.
  - projections are split into ~0.5-1.7us fill units (per-pair v units,
    512-col kT/qT/outproj halves) drained by deadline between attention
    ops so neither PE nor ACT ever starves for long.
"""

import numpy as np
import ml_dtypes
from contextlib import ExitStack

import concourse.bass as bass
import concourse.tile as tile
from concourse import bacc, mybir
from concourse.bass_utils import run_bass_kernel_spmd

L, N, D_IN = 4, 2048, 1024
H, DH = 8, 64
INNER = H * DH          # 512
D_OUT = D_IN
SCALE = DH ** -0.5      # 0.125
NQ = N // 2             # 1024 queries per core
NCORES = 8
DC = D_IN // 128        # 8 contraction chunks for the projections
NP = H // 2             # 4 head pairs
KC = N // 128           # 16 key chunks
LAG = 6                 # AV c-groups trail their scores by LAG key chunks
MASK_NEG = -50.0

BF = mybir.dt.bfloat16
F32 = mybir.dt.float32
EXP = mybir.ActivationFunctionType.Exp

# p-major phase order spreads the kT/qT/v fill deadlines across all phases
PHASES = [(p, qh) for p in range(NP) for qh in (0, 1)]


def _emit(ctx, tc, xT, wqT, wkT, wvT, woT, bo, maskb, ident, out):
    nc = tc.nc

    const = ctx.enter_context(tc.tile_pool(name="const", bufs=1))
    big = ctx.enter_context(tc.tile_pool(name="big", bufs=1))
    atp = ctx.enter_context(tc.tile_pool(name="atp", bufs=12))
    nrm = ctx.enter_context(tc.tile_pool(name="nrm", bufs=3))
    outp = ctx.enter_context(tc.tile_pool(name="outp", bufs=2))
    ps_st = ctx.enter_context(tc.tile_pool(name="ps_st", bufs=2, space="PSUM"))
    ps_ch = ctx.enter_context(tc.tile_pool(name="ps_ch", bufs=2, space="PSUM"))
    ps_fl = ctx.enter_context(tc.tile_pool(name="ps_fl", bufs=1, space="PSUM"))
    ps_av = ctx.enter_context(tc.tile_pool(name="ps_av", bufs=1, space="PSUM"))

    # ---- inputs -> SBUF
    wk_s = const.tile([128, DC, INNER], BF)
    wq_s = const.tile([128, DC, INNER], BF)
    wv_s = const.tile([128, DC, INNER], BF)
    wo_s = const.tile([128, NP, D_OUT], BF)
    bo_s = const.tile([1, D_OUT], F32)
    maskb_s = const.tile([128, KC], F32)
    id_s = const.tile([128, 128], BF)
    xT_s = big.tile([128, DC, N], BF)
    nc.sync.dma_start(wk_s, wkT)
    for d in range(DC):
        nc.sync.dma_start(xT_s[:, d, 0:512], xT[d][:, 0:512])
    nc.sync.dma_start(wq_s, wqT)
    nc.sync.dma_start(maskb_s, maskb)
    for d in range(DC):
        nc.sync.dma_start(xT_s[:, d, 512:NQ], xT[d][:, 512:NQ])
    nc.sync.dma_start(wv_s, wvT)
    for d in range(DC):
        nc.sync.dma_start(xT_s[:, d, NQ:N], xT[d][:, NQ:N])
    nc.sync.dma_start(wo_s, woT)
    nc.sync.dma_start(bo_s, bo)
    nc.sync.dma_start(id_s, ident)

    ones_row = const.tile([1, 128], BF)
    nc.vector.memset(ones_row, 1.0)
    bo_bf = const.tile([1, D_OUT], BF)
    nc.vector.tensor_copy(bo_bf, bo_s)

    kT_s = big.tile([128, NP, N], BF)
    qT_s = big.tile([128, NP, NQ], BF)
    v_sb = big.tile([128, KC, H, DH + 1], BF)
    nc.vector.memset(v_sb[:, :, :, DH], 1.0)
    o_sb = big.tile([128, 8, H, DH], BF)      # [q, t_global, h, dh]
    oT_sb = big.tile([128, NP, NQ], BF)       # [inner-of-pair, p, q]

    # ---- warmup: junk matmuls lift the PE HAM clock gate and a junk exp
    # pulls the ACT table load off the critical path, all during DMA.
    warm = const.tile([128, 512], BF)
    nc.vector.memset(warm, 1.0)
    wps = ps_av.tile([128, 512], F32, tag="av", name="wps")
    for i in range(18):
        nc.tensor.matmul(wps, warm[:, 0:128], warm, start=(i == 0),
                         stop=(i == 17))
    warm_out = const.tile([1, 32], BF)
    nc.scalar.activation(warm_out, wps[0:1, 0:32], EXP, bias=0.0, scale=0.0)

    # ---- fill units: lists of single-matmul closures + an evict closure.
    # Drained between attention ops a couple of matmuls at a time.
    def unit_kq(which, p, kh, half):
        w_s = wk_s if which == "k" else wq_s
        dst = kT_s if which == "k" else qT_s
        col = kh * 1024 + half * 512
        ps = [None]

        def mk(d):
            def f():
                if ps[0] is None:
                    ps[0] = ps_fl.tile([128, 512], F32, tag="fl",
                                       name=f"f{which}{p}{kh}{half}")
                nc.tensor.matmul(ps[0], w_s[:, d, p * 128:(p + 1) * 128],
                                 xT_s[:, d, col:col + 512],
                                 start=(d == 0), stop=(d == DC - 1))
            return f

        def ev():
            nc.vector.tensor_copy(dst[:, p, col:col + 512], ps[0])
        return [mk(d) for d in range(DC)], ev

    def unit_v(c, p):
        ps = [None]

        def mk(d):
            def f():
                if ps[0] is None:
                    ps[0] = ps_fl.tile([128, 512], F32, tag="fl",
                                       name=f"fv{c}{p}")
                nc.tensor.matmul(ps[0][:, 0:128],
                                 xT_s[:, d, c * 128:(c + 1) * 128],
                                 wv_s[:, d, p * 128:(p + 1) * 128],
                                 start=(d == 0), stop=(d == DC - 1))
            return f

        def ev():
            nc.vector.tensor_copy(
                v_sb[:, c, 2 * p:2 * p + 2, 0:DH],
                ps[0][:, 0:128].rearrange("k (h e) -> k h e", h=2))
        return [mk(d) for d in range(DC)], ev

    of_tiles = {}

    def unit_outproj(t, half):
        hs = slice(half * 512, (half + 1) * 512)
        ps = [None]

        def bias():
            if ps[0] is None:
                ps[0] = ps_fl.tile([128, 512], F32, tag="fl", name=f"fo{t}{half}")
            nc.tensor.matmul(ps[0], ones_row, bo_bf[:, hs],
                             start=True, stop=False)

        def mk(p):
            def f():
                nc.tensor.matmul(ps[0], oT_sb[:, p, t * 128:(t + 1) * 128],
                                 wo_s[:, p, hs], start=False,
                                 stop=(p == NP - 1))
            return f

        def ev():
            if t not in of_tiles:
                of_tiles[t] = outp.tile([128, D_OUT], F32, tag="of",
                                        name=f"of{t}")
            of = of_tiles[t]
            nc.vector.tensor_copy(of[:, hs], ps[0])
            if half == 1:
                nc.sync.dma_start(out[t * 128:(t + 1) * 128, :], of)
        return [bias] + [mk(p) for p in range(NP)], ev

    # fill queue: (deadline_slot, mms, evict); deadline in global c-slots
    fill_q = []

    def add_unit(deadline, unit):
        fill_q.append([deadline, unit[0], unit[1], 0])

    # deadline = slot whose scores/AV first READS the unit's output; the
    # drain forces full emission one slot before that.
    sched = []
    sched.append((0 * 16 + 4, unit_kq("k", 0, 0, 1)))
    sched.append((0 * 16 + 8, unit_kq("k", 0, 1, 0)))
    sched.append((0 * 16 + 12, unit_kq("k", 0, 1, 1)))
    for j in range(16):
        sched.append((0 * 16 + LAG + j, unit_v(j, 0)))
    sched.append((1 * 16 + 0, unit_kq("q", 0, 0, 1)))   # qh1 queries of p0
    for p in range(1, NP):
        base = 2 * p * 16
        sched.append((base - 4, unit_kq("q", p, 0, 0)))
        for kh in range(2):
            for half in range(2):
                sched.append((base + 8 * kh + 4 * half,
                              unit_kq("k", p, kh, half)))
        for j in range(16):
            sched.append((base + LAG + j, unit_v(j, p)))
        sched.append((base + 16, unit_kq("q", p, 0, 1)))
    for dl, u in sorted(sched, key=lambda x: x[0]):
        add_unit(dl, u)

    def emit_fill(slot, budget):
        # forced: units whose deadline is imminent; then opportunistic budget
        while fill_q:
            dl, mms, ev, idx = fill_q[0]
            forced = dl <= slot + 1
            if not forced and budget <= 0:
                break
            n = len(mms) - idx if forced else min(budget, len(mms) - idx)
            for i in range(idx, idx + n):
                mms[i]()
            budget -= n
            fill_q[0][3] = idx + n
            if fill_q[0][3] == len(mms):
                ev()
                fill_q.pop(0)
            if not forced and budget <= 0:
                break

    # ---- attention machinery
    at_tiles = [[None] * KC for _ in range(8)]
    av_chains = [None] * 8

    def scores(k, c):
        p, qh = PHASES[k]
        sT = ps_st.tile([128, 1024], F32, tag="st", name="sT")
        nc.tensor.matmul(
            sT[:, 0:512], kT_s[0:64, p, c * 128:(c + 1) * 128],
            qT_s[0:64, p, qh * 512:(qh + 1) * 512], start=True, stop=True)
        nc.tensor.matmul(
            sT[:, 512:1024], kT_s[64:128, p, c * 128:(c + 1) * 128],
            qT_s[64:128, p, qh * 512:(qh + 1) * 512], start=True, stop=True)
        att = atp.tile([128, 1024], BF, tag="at", name="at")
        nc.scalar.activation(att, sT, EXP, bias=maskb_s[:, c:c + 1],
                             scale=SCALE)
        at_tiles[k][c] = att

    def av_group(k, j):
        # chain (hl, tl) accumulates attn^T.T @ [v|1] for 128 queries
        p, qh = PHASES[k]
        avA, avB = av_chains[k]
        att = at_tiles[k][j]
        for hl in range(2):
            ps = avA if hl == 0 else avB
            for tl in range(4):
                # start zeroes the whole 2KB bank (all 4 chains); per-element
                # first-touch overwrites handle the other chains' first writes
                nc.tensor.matmul(
                    ps[:, tl * 65:tl * 65 + 65],
                    att[:, hl * 512 + tl * 128:hl * 512 + (tl + 1) * 128],
                    v_sb[:, j, 2 * p + hl, :],
                    start=(j == 0 and tl == 0),
                    stop=(j == KC - 1 and tl == 3))
        at_tiles[k][j] = None

    def norm_tr(k):
        # normalize by the accumulated denominators (col 64 of each chain)
        # and transpose O blocks into oT for the output projection
        p, qh = PHASES[k]
        avA, avB = av_chains[k]
        rA = nrm.tile([128, 4], F32, tag="rA", name="rA")
        rB = nrm.tile([128, 4], F32, tag="rB", name="rB")
        nc.vector.reciprocal(rA, avA.rearrange("q (t x) -> q t x", x=65)[:, :, 64])
        nc.vector.reciprocal(rB, avB.rearrange("q (t x) -> q t x", x=65)[:, :, 64])
        for tl in range(4):
            tg = qh * 4 + tl
            nc.vector.tensor_scalar_mul(
                o_sb[:, tg, 2 * p, :], avA[:, tl * 65:tl * 65 + 64],
                rA[:, tl:tl + 1])
            nc.vector.tensor_scalar_mul(
                o_sb[:, tg, 2 * p + 1, :], avB[:, tl * 65:tl * 65 + 64],
                rB[:, tl:tl + 1])
        trc = ps_av.tile([128, 512], BF, tag="av", name="trc")
        for tl in range(4):
            tg = qh * 4 + tl
            nc.tensor.transpose(trc[:, tl * 128:(tl + 1) * 128],
                                o_sb[:, tg, 2 * p:2 * p + 2, :], id_s)
        nc.vector.tensor_copy(oT_sb[:, p, qh * 512:(qh + 1) * 512], trc)

    # ---- start: minimal projections for phase 0 chunk 0
    ka, kev = unit_kq("k", 0, 0, 0)
    for f in ka:
        f()
    kev()
    qa, qev = unit_kq("q", 0, 0, 0)
    for f in qa:
        f()
    qev()

    # ---- main loop
    for k in range(8):
        p, qh = PHASES[k]
        avA = ps_ch.tile([128, 260], F32, tag="ch", name=f"avA{k}")
        avB = ps_ch.tile([128, 260], F32, tag="ch", name=f"avB{k}")
        av_chains[k] = (avA, avB)
        for c in range(KC):
            slot = k * 16 + c
            emit_fill(slot, 0)   # forced-by-deadline only, ahead of scores
            scores(k, c)
            if c < LAG and k >= 1:
                av_group(k - 1, KC - LAG + c)
                if c == LAG - 1:
                    norm_tr(k - 1)
                    emit_fill(slot, 2)
                    if k == 7:  # after norm of phase 6: qh0 outproj ready
                        for t in range(4):
                            for half in range(2):
                                add_unit(130, unit_outproj(t, half))
            if c - LAG >= 0:
                av_group(k, c - LAG)
            emit_fill(slot, 2 if c >= LAG else 3)

    # ---- tail: last phase's trailing AV groups, normalize, outproj qh=1
    for j in range(KC - LAG, KC):
        av_group(7, j)
        emit_fill(127, 2)
    norm_tr(7)
    while fill_q:
        emit_fill(10 ** 6, 16)
    for t in range(4, 8):
        for half in range(2):
            mms, ev = unit_outproj(t, half)
            for f in mms:
                f()
            ev()


def _build():
    nc = bacc.Bacc("TRN2", target_bir_lowering=False, debug=False,
                   num_devices=NCORES)
    aps = dict(
        xT=nc.dram_tensor("xT", [DC, 128, N], BF, kind="ExternalInput").ap(),
        wqT=nc.dram_tensor("wqT", [128, DC, INNER], BF, kind="ExternalInput").ap(),
        wkT=nc.dram_tensor("wkT", [128, DC, INNER], BF, kind="ExternalInput").ap(),
        wvT=nc.dram_tensor("wvT", [128, DC, INNER], BF, kind="ExternalInput").ap(),
        woT=nc.dram_tensor("woT", [128, NP, D_OUT], BF, kind="ExternalInput").ap(),
        bo=nc.dram_tensor("bo", [1, D_OUT], F32, kind="ExternalInput").ap(),
        maskb=nc.dram_tensor("maskb", [128, KC], F32, kind="ExternalInput").ap(),
        ident=nc.dram_tensor("ident", [128, 128], BF, kind="ExternalInput").ap(),
        out=nc.dram_tensor("out", [NQ, D_OUT], F32, kind="ExternalOutput").ap(),
    )
    with tile.TileContext(nc) as tc:
        with ExitStack() as ctx:
            _emit(ctx, tc, **aps)
    nc.compile()
    return nc


_prog = None


def _get_prog():
    global _prog
    if _prog is None:
        _prog = _build()
    return _prog


def _make_in_maps(x, Wq, Wk, Wv, Wo, bo, mask):
    bf = ml_dtypes.bfloat16
    f32 = np.float32

    def wlayout(w, chunks):
        # [out, in] -> partition-major [128, chunks, out]
        t = np.asarray(w).T.astype(bf).reshape(chunks, 128, w.shape[0])
        return np.ascontiguousarray(t.transpose(1, 0, 2))

    wqT = wlayout(Wq, DC)
    wkT = wlayout(Wk, DC)
    wvT = wlayout(Wv, DC)
    woT = wlayout(Wo, NP)
    bo2 = np.ascontiguousarray(bo).astype(f32).reshape(1, D_OUT)
    ident = np.eye(128, dtype=bf)
    in_maps = []
    for c in range(NCORES):
        l, qh = c // 2, c % 2
        # key order per core: own query half first (q proj reads cols 0..NQ)
        perm = np.r_[qh * NQ:(qh + 1) * NQ, (1 - qh) * NQ:(2 - qh) * NQ]
        xTl = np.ascontiguousarray(x[l][perm].T.astype(bf).reshape(DC, 128, N))
        mb = np.where(mask[l][perm], 0.0, MASK_NEG).astype(f32)
        mb = np.ascontiguousarray(mb.reshape(KC, 128).T)
        in_maps.append(dict(xT=xTl, wqT=wqT, wkT=wkT, wvT=wvT,
                            woT=woT, bo=bo2, maskb=mb, ident=ident))
    return in_maps


def run(x, Wq, Wk, Wv, Wo, bo, mask, trace=False, tmpdir=None):
    nc = _get_prog()
    in_maps = _make_in_maps(x, Wq, Wk, Wv, Wo, bo, mask)
    res = run_bass_kernel_spmd(nc, in_maps, core_ids=list(range(NCORES)),
                               trace=trace, tmpdir=tmpdir)
    out = np.empty((L, N, D_OUT), np.float32)
    for c in range(NCORES):
        l, qh = c // 2, c % 2
        out[l, qh * NQ:(qh + 1) * NQ, :] = res.results[c]["out"]
    return out, res


def kernel(x, Wq, Wk, Wv, Wo, bo, mask):
    out, _ = run(np.asarray(x, np.float32), np.asarray(Wq, np.float32),
                 np.asarray(Wk, np.float32), np.asarray(Wv, np.float32),
                 np.asarray(Wo, np.float32), np.asarray(bo, np.float32),
                 np.asarray(mask))
    return out
